# revision 6
# baseline (speedup 1.0000x reference)
"""Trainium2 Bass kernel for the Bahdanau-style band recurrence.

Math (per batch row b, position j, T=8 steps):
    g[j]   = W1 @ x[:, j] + b1 + b2                      (d=256)
    up[j]  <- relu(g[j] + W2 @ up[j-1])   (up[-1] = 0)
    dn[j]  <- relu(g[j] + W2 @ dn[j+1])   (dn[L]  = 0)
    miu[j] = relu(W3 @ x[:, j] + b3 + 2*b4 + W4 @ up[j-1] + W4 @ dn[j+1])

Implementation notes:
  - Data-parallel over batch: 16 rows -> 2 rows on each of 8 NeuronCores.
  - Weight preprocessing (transposes, bias folding, x||ones) happens on the
    host; the NEFF takes the processed arrays as inputs.
  - State layout: [d (2 partition-tiles of 128), token] in SBUF, with one
    zero guard column per batch row so the +-1 position shift is a plain
    column offset in the matmul rhs AP.
  - The affine g-term is folded into each step's PSUM accumulation as a
    K=5 matmul with rhs [x; ones] and lhsT [W1^T; b1+b2], so the per-step
    elementwise work is a single relu (PSUM -> SBUF).
  - Matmuls run as float32r (1 cycle/row PE rate vs 4 for float32). All
    fp32r-matmul inputs are produced by compute-engine copies (rounding),
    as walrus requires.
  - relu evacuation: up lane on VectorE, dn lane on ScalarE — keeps every
    matmul at <=1 semaphore wait (the fp32r LDWEIGHTS slot allows only 1).
"""

import sys

sys.path.insert(0, "/opt/trn_rl_repo")

import numpy as np

import concourse.bass as bass
import concourse.bacc as bacc
import concourse.mybir as mybir
import concourse.tile as tile
from concourse.bass_utils import run_bass_kernel_spmd

BS, DIMS, L, D, T = 16, 4, 2048, 256, 8
NCORES = 8
BSL = BS // NCORES          # batch rows per core
LP = L + 1                  # row span incl. one guard column
CH = 512                    # token chunk (one PSUM bank)
NCH = L // CH               # chunks per batch row
F32 = mybir.dt.float32
F32R = mybir.dt.float32r
RELU = mybir.ActivationFunctionType.Relu


def _build_nc():
    nc = bacc.Bacc("TRN2", target_bir_lowering=False, debug=False,
                   num_devices=NCORES)

    xe_d = nc.dram_tensor("xe", [BSL, 5, L], F32, kind="ExternalInput").ap()
    w2t_d = nc.dram_tensor("w2t", [D, D], F32, kind="ExternalInput").ap()
    w4t_d = nc.dram_tensor("w4t", [D, D], F32, kind="ExternalInput").ap()
    fs_d = nc.dram_tensor("folds", [5, D], F32, kind="ExternalInput").ap()
    ff_d = nc.dram_tensor("foldf", [5, D], F32, kind="ExternalInput").ap()
    out_d = nc.dram_tensor("out_loc", [BSL, D, L], F32, kind="ExternalOutput").ap()

    with tile.TileContext(nc) as tc:
        with (
            tc.tile_pool(name="const", bufs=1) as cpool,
            tc.tile_pool(name="state", bufs=1) as spool,
            tc.tile_pool(name="stage", bufs=4) as stpool,
            tc.tile_pool(name="psum", bufs=8, space="PSUM") as ppool,
        ):
            # ------- load weights: DMA -> f32 staging -> DVE rounding copy
            w2s = [cpool.tile([128, D], F32, name=f"w2s{k}") for k in range(2)]
            w4s = [cpool.tile([128, D], F32, name=f"w4s{k}") for k in range(2)]
            fss = cpool.tile([5, D], F32, name="fss")
            ffs = cpool.tile([5, D], F32, name="ffs")
            w2t = [cpool.tile([128, D], F32R, name=f"w2t{k}") for k in range(2)]
            w4t = [cpool.tile([128, D], F32R, name=f"w4t{k}") for k in range(2)]
            fold_s = cpool.tile([5, D], F32R, name="fold_s")
            fold_f = cpool.tile([5, D], F32R, name="fold_f")
            for kt in range(2):
                nc.sync.dma_start(w2s[kt][:, :], w2t_d[kt * 128:(kt + 1) * 128, :])
                nc.sync.dma_start(w4s[kt][:, :], w4t_d[kt * 128:(kt + 1) * 128, :])
                nc.vector.tensor_copy(w2t[kt][:, :], w2s[kt][:, :])
                nc.vector.tensor_copy(w4t[kt][:, :], w4s[kt][:, :])
            nc.sync.dma_start(fss[0:5, :], fs_d[:, :])
            nc.sync.dma_start(ffs[0:5, :], ff_d[:, :])
            nc.vector.tensor_copy(fold_s[0:5, :], fss[0:5, :])
            nc.vector.tensor_copy(fold_f[0:5, :], ffs[0:5, :])

            # ------- x rhs: [x rows 0..3; ones], rounded to f32r
            xstage = spool.tile([5, BSL * L], F32, name="xstage")
            rhs5 = spool.tile([5, BSL * L], F32R, name="rhs5")
            for b in range(BSL):
                nc.sync.dma_start(xstage[0:5, b * L:(b + 1) * L], xe_d[b])
            nc.vector.tensor_copy(rhs5[0:5, :], xstage[0:5, :])

            # ------- state buffers (f32r; guards zeroed by lane engine)
            # up token l of row b -> column b*LP + 1 + l (guard at b*LP)
            # dn token l of row b -> column b*LP + l (guard at b*LP + L)
            up = [[spool.tile([128, BSL * LP], F32R, name=f"up{dt}_{pp}")
                   for pp in range(2)] for dt in range(2)]
            dn = [[spool.tile([128, BSL * LP], F32R, name=f"dn{dt}_{pp}")
                   for pp in range(2)] for dt in range(2)]
            zcol = cpool.tile([128, 1], F32, name="zcol")
            nc.vector.memset(zcol[:, :], 0.0)
            for dt in range(2):
                for pp in range(2):
                    for b in range(BSL):
                        nc.vector.tensor_copy(
                            up[dt][pp][:, b * LP: b * LP + 1], zcol[:, :])
                        nc.scalar.copy(
                            dn[dt][pp][:, b * LP + L: b * LP + L + 1],
                            zcol[:, :])

            # ------- T recurrence steps
            for t in range(T):
                dstp = t % 2
                srcp = (t + 1) % 2
                for lane in range(2):           # 0 = up (DVE), 1 = dn (ACT)
                    buf = up if lane == 0 else dn
                    for ot in range(2):
                        for b in range(BSL):
                            for c in range(NCH):
                                pt = ppool.tile([128, CH], F32, name="mm")
                                nc.tensor.matmul(
                                    pt,
                                    fold_s[0:5, ot * 128:(ot + 1) * 128],
                                    rhs5[0:5, b * L + c * CH:
                                         b * L + (c + 1) * CH],
                                    start=True, stop=(t == 0))
                                if t > 0:
                                    for kt in range(2):
                                        base = b * LP + c * CH + lane
                                        nc.tensor.matmul(
                                            pt,
                                            w2t[kt][:, ot * 128:(ot + 1) * 128],
                                            buf[kt][srcp][:, base: base + CH],
                                            start=False, stop=(kt == 1))
                                wbase = b * LP + c * CH + (1 - lane)
                                dst = buf[ot][dstp][:, wbase: wbase + CH]
                                if lane == 0:
                                    nc.vector.tensor_scalar_max(dst, pt, 0.0)
                                else:
                                    nc.scalar.activation(dst, pt, RELU)

            # ------- final miu
            fsrc = (T - 1) % 2
            evac = 0
            for ot in range(2):
                for b in range(BSL):
                    for c in range(NCH):
                        pt = ppool.tile([128, CH], F32, name="mm")
                        nc.tensor.matmul(
                            pt,
                            fold_f[0:5, ot * 128:(ot + 1) * 128],
                            rhs5[0:5, b * L + c * CH: b * L + (c + 1) * CH],
                            start=True, stop=False)
                        for kt in range(2):
                            base = b * LP + c * CH
                            nc.tensor.matmul(
                                pt, w4t[kt][:, ot * 128:(ot + 1) * 128],
                                up[kt][fsrc][:, base: base + CH],
                                start=False, stop=False)
                        for kt in range(2):
                            base = b * LP + c * CH + 1
                            nc.tensor.matmul(
                                pt, w4t[kt][:, ot * 128:(ot + 1) * 128],
                                dn[kt][fsrc][:, base: base + CH],
                                start=False, stop=(kt == 1))
                        st = stpool.tile([128, CH], F32, name="ostage")
                        if evac % 2 == 0:
                            nc.vector.tensor_scalar_max(st, pt, 0.0)
                        else:
                            nc.scalar.activation(st, pt, RELU)
                        evac += 1
                        nc.sync.dma_start(
                            out_d[b, ot * 128:(ot + 1) * 128,
                                  c * CH:(c + 1) * CH], st)
    nc.compile()
    return nc


_NC_CACHE = None


def _get_nc():
    global _NC_CACHE
    if _NC_CACHE is None:
        _NC_CACHE = _build_nc()
    return _NC_CACHE


def _prep_host(inputs):
    """Host-side weight preprocessing -> per-core in_maps."""
    f = np.float32
    x = np.ascontiguousarray(inputs["x"], dtype=f)          # (16, 4, 2048)
    W1, b1 = inputs["W1"].astype(f), inputs["b1"].astype(f)
    W2, b2 = inputs["W2"].astype(f), inputs["b2"].astype(f)
    W3, b3 = inputs["W3"].astype(f), inputs["b3"].astype(f)
    W4, b4 = inputs["W4"].astype(f), inputs["b4"].astype(f)
    w2t = np.ascontiguousarray(W2.T)                        # (256, 256) [k, o]
    w4t = np.ascontiguousarray(W4.T)
    folds = np.ascontiguousarray(
        np.concatenate([W1.T, (b1 + b2)[None, :]], axis=0))  # (5, 256)
    foldf = np.ascontiguousarray(
        np.concatenate([W3.T, (b3 + 2.0 * b4)[None, :]], axis=0))
    ones = np.ones((BSL, 1, L), dtype=f)
    in_maps = []
    for c in range(NCORES):
        xe = np.ascontiguousarray(
            np.concatenate([x[c * BSL:(c + 1) * BSL], ones], axis=1))
        in_maps.append(dict(xe=xe, w2t=w2t, w4t=w4t,
                            folds=folds, foldf=foldf))
    return in_maps


def _run(inputs, trace=False):
    nc = _get_nc()
    in_maps = _prep_host(inputs)
    res = run_bass_kernel_spmd(nc, in_maps, core_ids=list(range(NCORES)),
                               trace=trace)
    parts = [res.results[c]["out_loc"] for c in range(NCORES)]
    full = np.concatenate(parts, axis=0)                 # (16, 256, 2048)
    out = np.ascontiguousarray(full.transpose(0, 2, 1))  # (16, 2048, 256)
    return out, res


def kernel(**inputs):
    out, _ = _run(inputs, trace=False)
    return out


if __name__ == "__main__":
    nc = _build_nc()
    print("build ok")


# revision 7
# speedup vs baseline: 1.2681x; 1.2681x over previous
"""Trainium2 Bass kernel for the Bahdanau-style band recurrence.

Math (per batch row b, position j, T=8 steps):
    g[j]   = W1 @ x[:, j] + b1 + b2                      (d=256)
    up[j]  <- relu(g[j] + W2 @ up[j-1])   (up[-1] = 0)
    dn[j]  <- relu(g[j] + W2 @ dn[j+1])   (dn[L]  = 0)
    miu[j] = relu(W3 @ x[:, j] + b3 + 2*b4 + W4 @ up[j-1] + W4 @ dn[j+1])

Implementation notes:
  - Data-parallel over batch: 16 rows -> 2 rows on each of 8 NeuronCores.
  - Weight preprocessing (transposes, bias folding, x||ones) happens on the
    host; the NEFF takes the processed arrays as inputs.
  - State layout: [d (2 partition-tiles of 128), token] in SBUF, with one
    zero guard column per batch row so the +-1 position shift is a plain
    column offset in the matmul rhs AP.
  - The affine g-term is folded into each step's PSUM accumulation as a
    K=5 matmul with rhs [x; ones] and lhsT [W1^T; b1+b2], so the per-step
    elementwise work is a single relu (PSUM -> SBUF).
  - Matmuls run as float32r (1 cycle/row PE rate vs 4 for float32). All
    fp32r-matmul inputs are produced by compute-engine copies (rounding),
    as walrus requires.
  - relu evacuation: up lane on VectorE, dn lane on ScalarE — keeps every
    matmul at <=1 semaphore wait (the fp32r LDWEIGHTS slot allows only 1).
"""

import sys

sys.path.insert(0, "/opt/trn_rl_repo")

import numpy as np

import concourse.bass as bass
import concourse.bacc as bacc
import concourse.mybir as mybir
import concourse.tile as tile
from concourse.bass_utils import run_bass_kernel_spmd

BS, DIMS, L, D, T = 16, 4, 2048, 256, 8
NCORES = 8
BSL = BS // NCORES          # batch rows per core
LP = L + 1                  # row span incl. one guard column
CH = 512                    # token chunk (one PSUM bank)
NCH = L // CH               # chunks per batch row
F32 = mybir.dt.float32
F32R = mybir.dt.float32r
BF16 = mybir.dt.bfloat16
RELU = mybir.ActivationFunctionType.Relu


def _build_nc():
    nc = bacc.Bacc("TRN2", target_bir_lowering=False, debug=False,
                   num_devices=NCORES)

    xe_d = nc.dram_tensor("xe", [BSL, 5, L], F32, kind="ExternalInput").ap()
    w2t_d = nc.dram_tensor("w2t", [D, D], F32, kind="ExternalInput").ap()
    w4t_d = nc.dram_tensor("w4t", [D, D], F32, kind="ExternalInput").ap()
    fs_d = nc.dram_tensor("folds", [5, D], F32, kind="ExternalInput").ap()
    ff_d = nc.dram_tensor("foldf", [5, D], F32, kind="ExternalInput").ap()
    out_d = nc.dram_tensor("out_loc", [BSL, D, L], F32, kind="ExternalOutput").ap()

    with tile.TileContext(nc) as tc:
        with (
            tc.tile_pool(name="const", bufs=1) as cpool,
            tc.tile_pool(name="state", bufs=1) as spool,
            tc.tile_pool(name="stage", bufs=4) as stpool,
            tc.tile_pool(name="psum", bufs=8, space="PSUM") as ppool,
        ):
            # ------- load weights: DMA -> f32 staging -> DVE rounding copy
            w2s = [cpool.tile([128, D], F32, name=f"w2s{k}") for k in range(2)]
            w4s = [cpool.tile([128, D], F32, name=f"w4s{k}") for k in range(2)]
            fss = cpool.tile([5, D], F32, name="fss")
            ffs = cpool.tile([5, D], F32, name="ffs")
            w2t = [cpool.tile([128, D], BF16, name=f"w2t{k}") for k in range(2)]
            w4t = [cpool.tile([128, D], BF16, name=f"w4t{k}") for k in range(2)]
            fold_s = cpool.tile([5, D], BF16, name="fold_s")
            fold_f = cpool.tile([5, D], BF16, name="fold_f")
            for kt in range(2):
                nc.sync.dma_start(w2s[kt][:, :], w2t_d[kt * 128:(kt + 1) * 128, :])
                nc.sync.dma_start(w4s[kt][:, :], w4t_d[kt * 128:(kt + 1) * 128, :])
                nc.vector.tensor_copy(w2t[kt][:, :], w2s[kt][:, :])
                nc.vector.tensor_copy(w4t[kt][:, :], w4s[kt][:, :])
            nc.sync.dma_start(fss[0:5, :], fs_d[:, :])
            nc.sync.dma_start(ffs[0:5, :], ff_d[:, :])
            nc.vector.tensor_copy(fold_s[0:5, :], fss[0:5, :])
            nc.vector.tensor_copy(fold_f[0:5, :], ffs[0:5, :])

            # ------- x rhs: [x rows 0..3; ones], rounded to f32r
            xstage = spool.tile([5, BSL * L], F32, name="xstage")
            rhs5 = spool.tile([5, BSL * L], BF16, name="rhs5")
            for b in range(BSL):
                nc.sync.dma_start(xstage[0:5, b * L:(b + 1) * L], xe_d[b])
            nc.vector.tensor_copy(rhs5[0:5, :], xstage[0:5, :])

            # ------- state buffers (f32r; guards zeroed by lane engine)
            # up token l of row b -> column b*LP + 1 + l (guard at b*LP)
            # dn token l of row b -> column b*LP + l (guard at b*LP + L)
            up = [[spool.tile([128, BSL * LP], BF16, name=f"up{dt}_{pp}")
                   for pp in range(2)] for dt in range(2)]
            dn = [[spool.tile([128, BSL * LP], BF16, name=f"dn{dt}_{pp}")
                   for pp in range(2)] for dt in range(2)]
            zcol = cpool.tile([128, 1], F32, name="zcol")
            nc.vector.memset(zcol[:, :], 0.0)
            for dt in range(2):
                for pp in range(2):
                    for b in range(BSL):
                        nc.vector.tensor_copy(
                            up[dt][pp][:, b * LP: b * LP + 1], zcol[:, :])
                        nc.scalar.copy(
                            dn[dt][pp][:, b * LP + L: b * LP + L + 1],
                            zcol[:, :])

            # ------- T recurrence steps
            for t in range(T):
                dstp = t % 2
                srcp = (t + 1) % 2
                for lane in range(2):           # 0 = up (DVE), 1 = dn (ACT)
                    buf = up if lane == 0 else dn
                    for ot in range(2):
                        for b in range(BSL):
                            for c in range(NCH):
                                pt = ppool.tile([128, CH], F32, name="mm")
                                nc.tensor.matmul(
                                    pt,
                                    fold_s[0:5, ot * 128:(ot + 1) * 128],
                                    rhs5[0:5, b * L + c * CH:
                                         b * L + (c + 1) * CH],
                                    start=True, stop=(t == 0))
                                if t > 0:
                                    for kt in range(2):
                                        base = b * LP + c * CH + lane
                                        nc.tensor.matmul(
                                            pt,
                                            w2t[kt][:, ot * 128:(ot + 1) * 128],
                                            buf[kt][srcp][:, base: base + CH],
                                            start=False, stop=(kt == 1))
                                wbase = b * LP + c * CH + (1 - lane)
                                dst = buf[ot][dstp][:, wbase: wbase + CH]
                                if lane == 0:
                                    nc.vector.tensor_scalar_max(dst, pt, 0.0)
                                else:
                                    nc.scalar.activation(dst, pt, RELU)

            # ------- final miu
            fsrc = (T - 1) % 2
            evac = 0
            for ot in range(2):
                for b in range(BSL):
                    for c in range(NCH):
                        pt = ppool.tile([128, CH], F32, name="mm")
                        nc.tensor.matmul(
                            pt,
                            fold_f[0:5, ot * 128:(ot + 1) * 128],
                            rhs5[0:5, b * L + c * CH: b * L + (c + 1) * CH],
                            start=True, stop=False)
                        for kt in range(2):
                            base = b * LP + c * CH
                            nc.tensor.matmul(
                                pt, w4t[kt][:, ot * 128:(ot + 1) * 128],
                                up[kt][fsrc][:, base: base + CH],
                                start=False, stop=False)
                        for kt in range(2):
                            base = b * LP + c * CH + 1
                            nc.tensor.matmul(
                                pt, w4t[kt][:, ot * 128:(ot + 1) * 128],
                                dn[kt][fsrc][:, base: base + CH],
                                start=False, stop=(kt == 1))
                        st = stpool.tile([128, CH], F32, name="ostage")
                        if evac % 2 == 0:
                            nc.vector.tensor_scalar_max(st, pt, 0.0)
                        else:
                            nc.scalar.activation(st, pt, RELU)
                        evac += 1
                        nc.sync.dma_start(
                            out_d[b, ot * 128:(ot + 1) * 128,
                                  c * CH:(c + 1) * CH], st)
    nc.compile()
    return nc


_NC_CACHE = None


def _get_nc():
    global _NC_CACHE
    if _NC_CACHE is None:
        _NC_CACHE = _build_nc()
    return _NC_CACHE


def _prep_host(inputs):
    """Host-side weight preprocessing -> per-core in_maps."""
    f = np.float32
    x = np.ascontiguousarray(inputs["x"], dtype=f)          # (16, 4, 2048)
    W1, b1 = inputs["W1"].astype(f), inputs["b1"].astype(f)
    W2, b2 = inputs["W2"].astype(f), inputs["b2"].astype(f)
    W3, b3 = inputs["W3"].astype(f), inputs["b3"].astype(f)
    W4, b4 = inputs["W4"].astype(f), inputs["b4"].astype(f)
    w2t = np.ascontiguousarray(W2.T)                        # (256, 256) [k, o]
    w4t = np.ascontiguousarray(W4.T)
    folds = np.ascontiguousarray(
        np.concatenate([W1.T, (b1 + b2)[None, :]], axis=0))  # (5, 256)
    foldf = np.ascontiguousarray(
        np.concatenate([W3.T, (b3 + 2.0 * b4)[None, :]], axis=0))
    ones = np.ones((BSL, 1, L), dtype=f)
    in_maps = []
    for c in range(NCORES):
        xe = np.ascontiguousarray(
            np.concatenate([x[c * BSL:(c + 1) * BSL], ones], axis=1))
        in_maps.append(dict(xe=xe, w2t=w2t, w4t=w4t,
                            folds=folds, foldf=foldf))
    return in_maps


def _run(inputs, trace=False):
    nc = _get_nc()
    in_maps = _prep_host(inputs)
    res = run_bass_kernel_spmd(nc, in_maps, core_ids=list(range(NCORES)),
                               trace=trace)
    parts = [res.results[c]["out_loc"] for c in range(NCORES)]
    full = np.concatenate(parts, axis=0)                 # (16, 256, 2048)
    out = np.ascontiguousarray(full.transpose(0, 2, 1))  # (16, 2048, 256)
    return out, res


def kernel(**inputs):
    out, _ = _run(inputs, trace=False)
    return out


if __name__ == "__main__":
    nc = _build_nc()
    print("build ok")
